# revision 45
# baseline (speedup 1.0000x reference)
"""APPNP forward on 8 Trainium2 NeuronCores — dma_gather design (v3).

Reference: h = features; 10x: h = 0.9 * (segment_sum((h*ns)[src] by dst) * nd)
+ 0.1 * h0.  Nodes sharded 8 ways by dst (12544 rows/core); edges partitioned
by dst core.

Per core, per step:
  1. The scaled global table T = (h*ns) lives in device DRAM as
     [100352, 128] bf16 (cols 64:128 zero).  All 8 cores share one trn2
     device's HBM, so the per-step AllGather that rebuilds T is cheap
     (~12us measured).
  2. Edges are sorted by (bank, window, dst_local): bank = src//25088 (so
     gather indices fit int16), window = 128 consecutive dst rows.  Each
     (window, bank) run is padded to a multiple of 128 slots with idx-0 /
     dstw=-1 pads; per-(w,b) lengths are the max over cores so the
     instruction schedule is shared SPMD.
  3. dma_gather (SWDGE) fetches 896 edges per instruction from T's bank
     slice into SBUF tiles [128, 7, 128] bf16 (slot s -> partition s%128,
     col s//128).  Gathers round-robin over 4 SWDGE queues (desc-gen runs
     on distinct Q7 pairs) and ~24 tile buffers keep the pipeline deep.
     Measured ~1.18 ns/idx.
  4. Scatter: per 128-slot chunk one PE matmul accumulates into the
     (bank, window) psum tile [128, 64] fp32: lhsT = one-hot [128, C]
     (DVE is_equal of a per-chunk dstw column vs iota), rhs = the gather
     tile's col [128, 64].  First/last chunk of each run are full-width
     (start=True / stop=True) so every psum row is zeroed and closed; all
     other chunks are narrow col ranges with start=False.
  5. Bank passes 0-2 fold psum into an SBUF fp32 accumulator
     [128, 98, 64]; pass 3 fuses the fold with the blend
     h' = a*agg + b (a = .9*nd*ns, b = .1*h0*ns folded per-window), writes
     bf16 into a [128, G, 128] stage (cols 64:128 memset 0), DMAs to tin,
     and after the last window one AllGather rebuilds T.  The final step
     blends with a2 = .9*nd, b2 = .1*h0 into fp32 and writes `out`.
"""

import os
import sys

sys.path.insert(0, "/opt/trn_rl_repo")

import numpy as np
import ml_dtypes

BF16 = ml_dtypes.bfloat16

K_LAYERS = 10
ALPHA = 0.1
N_NODES = 100_000
D_FEAT = 64
M_CORES = 8

SHARD = 12544
NPAD = SHARD * M_CORES      # 100352
W = 128                     # window width (psum partitions)
NW = SHARD // W             # 98 windows/core
NBANKS = 4
BANK = NPAD // NBANKS       # 25088 rows per bank (int16-safe)
LG = int(os.environ.get("APPNP_LG", "896"))   # idxs per dma_gather
GCOLS = LG // 128
NQUEUE = 4
SCRATCH = int(os.environ.get("APPNP_SCRATCH", "16384"))
GBUF = int(os.environ.get("APPNP_GBUF", "56"))
G_STG = 7                   # windows per output stage group
NSTG = NW // G_STG          # 14 stage groups


def _preprocess(src, dst, M=M_CORES):
    """Build the shared slot schedule + per-core idx/dstw arrays."""
    E = src.shape[0]
    core = dst // SHARD
    ldst = dst - core * SHARD
    w = ldst // W
    dw = (ldst - w * W).astype(np.int64)
    b = src // BANK
    bi = (src - b * BANK).astype(np.int64)

    # run lengths: shared = ceil(max_c cnt / 128)*128   (>=128 for b==0)
    key = ((core * NBANKS + b) * NW + w)
    cnt = np.bincount(key, minlength=M * NBANKS * NW).reshape(M, NBANKS, NW)
    mx = cnt.max(axis=0)                       # [NBANKS, NW]
    L_run = ((mx + 127) // 128) * 128
    # every (bank, window) run must exist: group-psum folds read all 7
    # windows' col slices, so every slice needs its open/close matmuls
    L_run = np.maximum(L_run, 128)

    # bank segment starts (slots), segments padded to x896
    seg_len = L_run.sum(axis=1)                       # [NBANKS]
    seg_len_pad = ((seg_len + LG - 1) // LG) * LG
    seg_off = np.concatenate([[0], np.cumsum(seg_len_pad)])
    L_tot = int(seg_off[-1])

    # run start slot (global): bank offset + prefix of runs within bank
    run_start = np.zeros((NBANKS, NW), dtype=np.int64)
    for bb in range(NBANKS):
        run_start[bb] = seg_off[bb] + np.concatenate(
            [[0], np.cumsum(L_run[bb])[:-1]])

    # place edges: rank within (core, b, w) sorted by (dw, arrival)
    okey = ((core * NBANKS + b) * NW + w) * W + dw
    order = np.argsort(okey, kind="stable")
    oc, ob, ow = core[order], b[order], w[order]
    obi, odw = bi[order], dw[order]
    ckey = (oc * NBANKS + ob) * NW + ow
    starts = np.concatenate(
        [[0], np.cumsum(np.bincount(ckey, minlength=M * NBANKS * NW))])
    rank = np.arange(E) - starts[ckey]
    slot = run_start[ob, ow] + rank                   # [E]

    # per-core idx (bank-relative) and dstw arrays over the full slot space.
    # Pad slots forward-fill the preceding real index: a constant pad (e.g.
    # row 0) funnels ~25% of descriptors at one HBM row and measurably
    # throttles the gather DMA (~2.6x).
    idx_full = np.zeros((M, L_tot), dtype=np.int16)
    dstw_full = np.full((M, L_tot), -1.0, dtype=np.float32)
    idx_full[oc, slot] = obi.astype(np.int16)
    dstw_full[oc, slot] = odw
    real = dstw_full >= 0
    pos = np.where(real, np.arange(L_tot)[None, :], 0)
    ff = np.maximum.accumulate(pos, axis=1)
    idx_full = np.take_along_axis(idx_full, ff, axis=1)

    # chunks: 128-slot groups inside runs (skip pure inter-segment pad tail).
    # all-pad middle chunks (no real slot in ANY core) are dropped: their
    # matmuls contribute zero and first/last already open/close the psum.
    # chunk meta: (bank, w, gather_idx, col, first, last, s)
    has_edge_full = None  # set below
    chunks = []
    for bb in range(NBANKS):
        for ww in range(NW):
            s0 = run_start[bb, ww]
            nch = int(L_run[bb, ww]) // 128
            for k in range(nch):
                s = int(s0) + 128 * k
                chunks.append([bb, ww, s // LG, (s % LG) // 128,
                               k == 0, k == nch - 1, s])
    nchunks = len(chunks)

    # shared col ranges [a, b) per chunk from union of real slots, quantized
    # to legal PE out-tile positions: 32-aligned C=32, 64-aligned C=64, or
    # full 128.  First/last chunks of a run are full-width (psum open/close).
    has_edge = dstw_full >= 0
    lo = np.full(nchunks, 0, dtype=np.int64)
    hi = np.full(nchunks, W, dtype=np.int64)
    drop = []
    for i, (bb, ww, gi, col, first, last, s) in enumerate(chunks):
        if first or last:
            continue
        m = has_edge[:, s:s + 128]
        if not m.any():
            lo[i], hi[i] = 0, 0        # dropped: emit no matmul, no oh cols
            drop.append(i)
            continue
        blk = dstw_full[:, s:s + 128]
        a = int(blk[m].min())
        bcol = int(blk[m].max()) + 1
        a32 = a // 32 * 32
        if a32 < 96 and bcol <= a32 + 32:       # base 96 rejected by bass AP
            lo[i], hi[i] = a32, a32 + 32
            continue
        a64 = a // 64 * 64
        if bcol <= a64 + 64:
            lo[i], hi[i] = a64, a64 + 64
        else:
            lo[i], hi[i] = 0, W

    # precomputed one-hot blocks, variable width, packed in chunk order:
    # chunk i occupies oh cols [coff[i], coff[i]+C_i)
    coff = np.concatenate([[0], np.cumsum(hi - lo)]).astype(np.int64)
    totC = int(coff[-1])
    oh_all = np.zeros((M, 128, totC), dtype=BF16)
    for i, (bb, ww, gi, col, first, last, s) in enumerate(chunks):
        dwv = dstw_full[:, s:s + 128]                 # [M, 128]
        C = int(hi[i] - lo[i])
        rel = dwv - lo[i]
        valid = (rel >= 0) & (rel < C) & (dwv >= 0)
        m_idx, p_idx = np.nonzero(valid)
        oh_all[m_idx, p_idx,
               int(coff[i]) + rel[valid].astype(np.int64)] = 1.0

    # gathers: piece j covers slots [j*LG, (j+1)*LG); queue = j % NQUEUE
    # (bank never changes inside a piece by construction)
    ngather = L_tot // LG
    g_bank = np.zeros(ngather, dtype=np.int64)
    for j in range(ngather):
        g_bank[j] = np.searchsorted(seg_off[1:], j * LG, side="right")

    # idx tile: [128, words] int16; gather j (queue q=j%4, rank r=j//4)
    # occupies partitions [32q, 32q+32) words [56r, 56r+56), idxs wrapped
    # in 16 partitions, duplicated across the pair's two 16-part groups.
    nrank = (ngather + NQUEUE - 1) // NQUEUE
    words = LG // 16
    idx_t = np.zeros((M, 128, nrank * words), dtype=np.int16)
    for j in range(ngather):
        q, r = j % NQUEUE, j // NQUEUE
        blk = idx_full[:, j * LG:(j + 1) * LG].reshape(M, words, 16)
        blk = blk.transpose(0, 2, 1)                  # [M, 16, words]
        for half in range(2):
            p0 = 32 * q + 16 * half
            idx_t[:, p0:p0 + 16, r * words:(r + 1) * words] = blk
    # chunk index range per gather tile (chunks are in slot order)
    ti0 = np.full(ngather, nchunks, dtype=np.int64)
    ti1 = np.zeros(ngather, dtype=np.int64)
    for i, (bb, ww, gi, col, first, last, s) in enumerate(chunks):
        ti0[gi] = min(ti0[gi], i)
        ti1[gi] = max(ti1[gi], i + 1)

    return dict(
        L_tot=L_tot, chunks=chunks, lo=lo, hi=hi, nchunks=nchunks,
        ngather=ngather, g_bank=g_bank, nrank=nrank,
        idx_t=idx_t, oh_all=oh_all, coff=coff, totC=totC,
        ti0=ti0, ti1=ti1, L_run=L_run, run_start=run_start,
    )


def _dma_gather_narrow(gp, out_ap, in_ap, idxs_ap, num_idxs, elem_size,
                       queue_num=0):
    """dma_gather with elem_size_bytes < 256 (row STRIDE must still be a
    multiple of 256B).  Replicates bass.BassGpSimd.dma_gather minus the
    `elem_size_bytes % 256 == 0` assert, which only the transpose=True RX
    path actually needs (TRANSPOSE_DESC_BYTES); the non-transpose ucode
    reads `elem_size_bytes` per descriptor at `stride_bytes_256*256` row
    stride."""
    from concourse import mybir
    from concourse import ap_utils
    from concourse.bass import exact_div

    gp._assert_queue_num(queue_num)
    assert idxs_ap.dtype == mybir.dt.int16
    assert in_ap.dtype == out_ap.dtype
    elem_step = in_ap.ap[0][0]
    assert ap_utils.ap_is_contiguous(out_ap.ap[1:])
    assert ap_utils.ap_is_contiguous(idxs_ap.ap[1:])
    assert in_ap.ap[-1][1] == out_ap.ap[-1][1] == elem_size
    stride_bytes = elem_step * mybir.dt.size(in_ap.dtype)
    stride_bytes_256 = exact_div(stride_bytes, 256)
    _in_ap = gp.lower_ap_dma(in_ap, for_custom_bir_dma=True)
    _idxs_ap = gp.lower_ap(idxs_ap)
    _out_ap = gp.lower_ap(out_ap)
    return gp.add_instruction(
        mybir.InstDMAGatherAnt(
            name=gp.bass.get_next_instruction_name(),
            ins=[*_in_ap, _idxs_ap, gp.lower_val_access(gp.to_reg(num_idxs))],
            outs=[_out_ap],
            transpose=False,
            num_idxs=num_idxs,
            elem_size=elem_size,
            stride_bytes_256=stride_bytes_256,
            gen_mode=0,
            single_packet=True,
            queue_num=queue_num,
            sbuf_tokens_per_rank=0,
            sbuf_free_dim_per_rank=0,
            sbuf_free_dim_pad_per_rank=0,
            sbuf_byte_offset=0,
        )
    )


def _build_nc(meta, M, D, steps):
    import os
    from concourse import bass, bacc, tile, mybir

    ablate = os.environ.get("APPNP_ABLATE", "")
    dt = mybir.dt
    chunks, lo, hi = meta["chunks"], meta["lo"], meta["hi"]
    nchunks, ngather = meta["nchunks"], meta["ngather"]
    g_bank, nrank = meta["g_bank"], meta["nrank"]
    coff, totC = meta["coff"], meta["totC"]
    ti0, ti1 = meta["ti0"], meta["ti1"]
    words = LG // 16

    nc = bacc.Bacc("TRN2", target_bir_lowering=False, debug=False,
                   num_devices=M, num_swdge_queues=NQUEUE,
                   dynamic_dma_scratch_size=SCRATCH)

    tt0 = nc.dram_tensor("tt0", [NPAD, 128], dt.bfloat16,
                         kind="ExternalInput").ap()
    idx_d = nc.dram_tensor("idx", [128, nrank * words], dt.int16,
                           kind="ExternalInput").ap()
    oh_d = nc.dram_tensor("oh", [128, totC], dt.bfloat16,
                          kind="ExternalInput").ap()
    a_d = nc.dram_tensor("acoef", [W, NW], dt.float32, kind="ExternalInput").ap()
    b_d = nc.dram_tensor("bcoef", [W, NW * D], dt.bfloat16,
                         kind="ExternalInput").ap()
    a2_d = nc.dram_tensor("acoef2", [W, NW], dt.float32,
                          kind="ExternalInput").ap()
    b2_d = nc.dram_tensor("bcoef2", [W, NW * D], dt.float32,
                          kind="ExternalInput").ap()
    out = nc.dram_tensor("out", [SHARD, D], dt.float32,
                         kind="ExternalOutput").ap()

    with tile.TileContext(nc) as tc:
        with (
            tc.tile_pool(name="dram", bufs=1, space="DRAM") as dram,
            tc.tile_pool(name="const", bufs=1) as const,
            tc.tile_pool(name="aggp", bufs=1) as aggp,
            tc.tile_pool(name="gbuf", bufs=GBUF) as gbuf,
            tc.tile_pool(name="ohp", bufs=14) as ohp,
            tc.tile_pool(name="psa", bufs=8, space="PSUM") as psa,
            tc.tile_pool(name="tmpp", bufs=8) as tmpp,
            tc.tile_pool(name="stgp", bufs=4) as stgp,
        ):
            tt = dram.tile([NPAD, 128], dt.bfloat16, tag="tt", name="tt")
            tin = dram.tile([SHARD, 128], dt.bfloat16, tag="tin", name="tin")

            idx_t = const.tile_from(idx_d)
            if ablate != "gmin":
                a_t = const.tile_from(a_d)
                b_t = const.tile_from(b_d)
                a2_t = const.tile_from(a2_d)
                b2_t = const.tile_from(b2_d)

                aggbuf = aggp.tile([128, NW, D], dt.float32, tag="agg",
                                   name="aggbuf")

            # initial table load (in 4 pieces to keep descriptors sane)
            for p in range(4):
                r0 = NPAD // 4 * p
                r1 = NPAD // 4 * (p + 1)
                nc.sync.dma_start(out=tt[r0:r1, :], in_=tt0[r0:r1, :])

            for step in range(steps):
                last = step == steps - 1

                # ---- gathers (round-robin queues, deep buffer rotation) ----
                gtiles = []
                for j in range(ngather):
                    q, r = j % NQUEUE, j // NQUEUE
                    bb = int(g_bank[j])
                    gt = gbuf.tile([128, GCOLS, D], dt.bfloat16, tag="gt",
                                   name="gt")
                    _dma_gather_narrow(
                        nc.gpsimd,
                        gt[:, :, :],
                        tt[bb * BANK:(bb + 1) * BANK, 0:D],
                        idx_t[:, r * words:(r + 1) * words],
                        LG, D,
                        queue_num=q,
                    )
                    gtiles.append(gt)

                if ablate == "gmin":
                    continue
                if ablate.startswith("gather"):
                    # drain pipeline: write a dummy tin so the AG still fires
                    zstg = stgp.tile([W, G_STG, 128], dt.bfloat16, tag="stg",
                                     name="stg")
                    nc.vector.memset(zstg[:, :, :], 0.0)
                    nc.sync.dma_start(
                        out=tin[0:G_STG * W, :]
                            .rearrange("(a p) d -> p a d", p=W),
                        in_=zstg[:, :, :])
                    if not last and ablate != "gathernoag":
                        nc.gpsimd.collective_compute(
                            "AllGather", mybir.AluOpType.bypass,
                            replica_groups=[list(range(M))],
                            ins=[tin[:, :].opt()], outs=[tt[:, :].opt()])
                    continue

                # ---- scatter / fold / blend, in slot order ----
                maxCW = int(max(
                    coff[ti1[j]] - coff[ti0[j]]
                    for j in range(ngather) if ti1[j] > ti0[j]))
                aggps = None
                stg = None
                stg_f = None
                ohblk = None
                c0 = 0
                for i, (bb, ww, gi, col, first, lastc, s) in enumerate(chunks):
                    a, bcol = int(lo[i]), int(hi[i])
                    if i == int(ti0[gi]):
                        c0 = int(coff[ti0[gi]])
                        c1 = int(coff[ti1[gi]])
                        if c1 > c0:
                            ohblk = ohp.tile([128, maxCW], dt.bfloat16,
                                             tag="ohb", name="ohb")
                            nc.scalar.dma_start(out=ohblk[:, :c1 - c0],
                                                in_=oh_d[:, c0:c1])
                    if ablate == "nomm":
                        continue
                    if first:
                        aggps = psa.tile([W, D], dt.float32, tag="ps",
                                         name="ps")
                    if a < bcol:
                        o0 = int(coff[i]) - c0
                        o1 = int(coff[i + 1]) - c0
                        nc.tensor.matmul(
                            out=aggps[a:bcol, :],
                            lhsT=ohblk[:, o0:o1],
                            rhs=gtiles[gi][:, col, 0:D],
                            start=bool(first), stop=bool(lastc),
                        )
                    if ablate == "nofold":
                        continue
                    if not lastc:
                        continue

                    # run (bb, ww) closed -> fold or blend
                    if bb < NBANKS - 1:
                        if bb == 0:
                            nc.vector.tensor_copy(out=aggbuf[:, ww, :],
                                                  in_=aggps[:, :])
                        else:
                            nc.vector.tensor_tensor(
                                out=aggbuf[:, ww, :], in0=aggps[:, :],
                                in1=aggbuf[:, ww, :],
                                op=mybir.AluOpType.add,
                            )
                        continue

                    # pass 3: fused fold + blend per window
                    wi = ww % G_STG
                    if wi == 0:
                        if last:
                            stg_f = stgp.tile([W, G_STG, D], dt.float32,
                                              tag="stgf", name="stg_f")
                        else:
                            stg = stgp.tile([W, G_STG, 128], dt.bfloat16,
                                            tag="stg", name="stg")
                            nc.vector.memset(stg[:, :, D:], 0.0)
                    tmp = tmpp.tile([W, D], dt.float32, tag="tmp", name="tmp")
                    nc.vector.tensor_tensor(
                        out=tmp[:, :], in0=aggps[:, :], in1=aggbuf[:, ww, :],
                        op=mybir.AluOpType.add,
                    )
                    ca = (a2_t if last else a_t)[:, ww:ww + 1]
                    cb = (b2_t if last else b_t)[:, ww * D:(ww + 1) * D]
                    tmp2 = tmpp.tile([W, D], dt.float32, tag="tmp2",
                                     name="tmp2")
                    nc.vector.tensor_scalar(
                        out=tmp2[:, :], in0=tmp[:, :], scalar1=ca,
                        scalar2=None, op0=mybir.AluOpType.mult,
                    )
                    nc.vector.tensor_tensor(
                        out=(stg_f[:, wi, :] if last else stg[:, wi, 0:D]),
                        in0=tmp2[:, :], in1=cb,
                        op=mybir.AluOpType.add,
                    )
                    if wi == G_STG - 1:
                        w0 = ww - G_STG + 1
                        if last:
                            nc.sync.dma_start(
                                out=out[w0 * W:(w0 + G_STG) * W, :]
                                    .rearrange("(a p) d -> p a d", p=W),
                                in_=stg_f[:, :, :],
                            )
                        else:
                            nc.sync.dma_start(
                                out=tin[w0 * W:(w0 + G_STG) * W, :]
                                    .rearrange("(a p) d -> p a d", p=W),
                                in_=stg[:, :, :],
                            )

                if not last:
                    import os
                    if os.environ.get("APPNP_NOAG"):
                        # TimelineSim can't model collectives; substitute
                        # local DMAs with the same write volume.
                        for c in range(M):
                            nc.sync.dma_start(
                                out=tt[c * SHARD:(c + 1) * SHARD, :],
                                in_=tin[:, :])
                    else:
                        nc.gpsimd.collective_compute(
                            "AllGather",
                            mybir.AluOpType.bypass,
                            replica_groups=[list(range(M))],
                            ins=[tin[:, :].opt()],
                            outs=[tt[:, :].opt()],
                        )
    nc.compile()
    return nc


def _make_inputs(features, src, dst, meta, M, D, alpha):
    n = features.shape[0]

    deg_out = np.bincount(src, minlength=n).astype(np.float32)
    deg_in = np.bincount(dst, minlength=n).astype(np.float32)
    ns = np.clip(deg_out, 1.0, None) ** -0.5
    nd = np.clip(deg_in, 1.0, None) ** -0.5

    ns_pad = np.ones(NPAD, dtype=np.float32)
    nd_pad = np.ones(NPAD, dtype=np.float32)
    h0_pad = np.zeros((NPAD, D), dtype=np.float32)
    ns_pad[:n] = ns
    nd_pad[:n] = nd
    h0_pad[:n] = features

    tt0 = np.zeros((NPAD, 128), dtype=BF16)
    tt0[:, :D] = (h0_pad * ns_pad[:, None]).astype(BF16)

    def wmaj(x):
        x = x.reshape(NW, W, -1).transpose(1, 0, 2)
        return np.ascontiguousarray(x.reshape(W, -1))

    in_maps = []
    for c in range(M):
        sl = slice(c * SHARD, (c + 1) * SHARD)
        a = ((1.0 - alpha) * nd_pad[sl] * ns_pad[sl]).astype(np.float32)
        b = (alpha * h0_pad[sl] * ns_pad[sl][:, None]).astype(np.float32)
        a2 = ((1.0 - alpha) * nd_pad[sl]).astype(np.float32)
        b2 = (alpha * h0_pad[sl]).astype(np.float32)
        in_maps.append({
            "tt0": tt0,
            "idx": meta["idx_t"][c],
            "oh": meta["oh_all"][c],
            "acoef": wmaj(a).astype(np.float32),
            "bcoef": wmaj(b).astype(BF16),
            "acoef2": wmaj(a2).astype(np.float32),
            "bcoef2": wmaj(b2).astype(np.float32),
        })
    return in_maps


_NC_CACHE = {}


def build_all(features, src, dst, *, M=M_CORES, D=D_FEAT,
              steps=K_LAYERS, alpha=ALPHA, **kw):
    src = np.asarray(src).astype(np.int64)
    dst = np.asarray(dst).astype(np.int64)
    meta = _preprocess(src, dst, M)
    key = (meta["L_tot"], meta["nchunks"], steps, M)
    if key not in _NC_CACHE:
        _NC_CACHE[key] = _build_nc(meta, M, D, steps)
    nc = _NC_CACHE[key]
    in_maps = _make_inputs(np.asarray(features, np.float32), src, dst,
                           meta, M, D, alpha)
    return nc, in_maps, meta


def kernel(features, src, dst, *, trace=False, **kw):
    from concourse.bass_utils import run_bass_kernel_spmd

    features = np.asarray(features)
    nc, in_maps, meta = build_all(features, src, dst, **kw)
    res = run_bass_kernel_spmd(nc, in_maps, core_ids=list(range(M_CORES)),
                               trace=trace)
    n = features.shape[0]
    h = np.concatenate([res.results[c]["out"] for c in range(M_CORES)], axis=0)
    out = np.ascontiguousarray(h[:n]).astype(np.float32)
    if trace:
        kernel.last_results = res
    return out
